# revision 21
# baseline (speedup 1.0000x reference)
"""BERT self-attention (B=4, S=2048, H=768, 12 heads x 64) on 8 trn2 cores.

Sharding: core c = batch (c//2) x head-half (c%2, 6 heads each).
Each core computes Q/K/V projections for its 6 heads, attention, and a
partial output projection (its heads' slice of Wo). Host sums the two
partials per batch and adds bo.

On-device layout (per core):
  xT   [768, 2048]  bf16  (DMA-transposed x)
  QT/KT per head-pair [128=2x64, 2048] bf16   (head-dim on partitions)
  V    16 tiles [128 keys, 6 heads x 65] bf16 (col 64 of each head = 1.0)
  scores^T [128 keys, 2x512 q] fp32 PSUM (two heads packed via row tiling)
  exp on ScalarE (scale=1/8, bias=mask column), out bf16
  attn@V -> comb [65, 512] PSUM; row 64 = softmax denominator
  combT_A/B [64, 2048] bf16, scaled by 1/denom
  out-proj: 6 x K=64 chunks accumulated in SBUF out_acc, fp32

The attention loop is ACT(exp)-bound. The in-order PE is kept busy by
(a) pipelining the score matmuls one iteration ahead (across sq and
head-pair boundaries), and (b) injecting independent projection matmuls
(next pair's Q/K proj, previous tiles' out-proj) into the exp-wait
bubble between scores(i+1) and attn@V(i).
"""

import numpy as np
import ml_dtypes

B, S, H = 4, 2048, 768
NH, HS = 12, 64
NHL = 6              # heads per core
NHP = 3              # head pairs per core
HCHUNKS = 6          # 768 / 128 contraction chunks
SKT = 16             # key tiles of 128
SQT = 4              # query tiles of 512
QW = 512             # query tile width
N_CORES = 8

_COMPILED = None


def _build():
    import concourse.bass as bass
    import concourse.mybir as mybir
    import concourse.tile as tile
    from concourse import bacc

    fp32 = mybir.dt.float32
    bf16 = mybir.dt.bfloat16
    AF = mybir.ActivationFunctionType

    nc = bacc.Bacc("TRN2", target_bir_lowering=False, debug=False)

    xt_d = nc.dram_tensor("xt", [H, S], bf16, kind="ExternalInput").ap()
    wq_d = nc.dram_tensor("wq", [H, NHL * HS], bf16, kind="ExternalInput").ap()
    wk_d = nc.dram_tensor("wk", [H, NHL * HS], bf16, kind="ExternalInput").ap()
    wv_d = nc.dram_tensor("wv", [H, NHL * HS], bf16, kind="ExternalInput").ap()
    wo_d = nc.dram_tensor("wo", [NHL * HS, H], bf16, kind="ExternalInput").ap()
    bq_d = nc.dram_tensor("bq", [128, NHP], fp32, kind="ExternalInput").ap()
    bk_d = nc.dram_tensor("bk", [128, NHP], fp32, kind="ExternalInput").ap()
    bv_d = nc.dram_tensor("bv", [128, NHL * HS], fp32, kind="ExternalInput").ap()
    mask_d = nc.dram_tensor("mask", [128, SKT], fp32, kind="ExternalInput").ap()
    out_d = nc.dram_tensor("out", [S, H], fp32, kind="ExternalOutput").ap()

    with tile.TileContext(nc) as tc:
        with (
            tc.tile_pool(name="const", bufs=1) as const,
            tc.tile_pool(name="xt", bufs=1) as xtp,
            tc.tile_pool(name="vsb", bufs=1) as vsb,
            tc.tile_pool(name="qkt", bufs=2) as qkt,
            tc.tile_pool(name="combt", bufs=1) as combtp,
            tc.tile_pool(name="oacc", bufs=1) as oaccp,
            tc.tile_pool(name="attn", bufs=4) as attnp,
            tc.tile_pool(name="small", bufs=4) as smallp,
            tc.tile_pool(name="ps_sc", bufs=2, space="PSUM") as ps_sc,
            tc.tile_pool(name="ps_cb", bufs=2, space="PSUM") as ps_cb,
            tc.tile_pool(name="ps_pj", bufs=2, space="PSUM") as ps_pj,
        ):
            # ---- x^T (host-transposed) on the sync HWDGE queue, weights
            # in parallel on the scalar HWDGE queue ----
            # x^T as separate [128, 512] piece-tiles so the first V/K
            # chains start as soon as the leading pieces have landed.
            # All startup DMAs are emitted in consumption order, alternating
            # between the two HWDGE queues.
            _dma_i = [0]

            def ld(dst, srcap):
                eng = nc.sync if _dma_i[0] % 2 == 0 else nc.scalar
                _dma_i[0] += 1
                eng.dma_start(dst, srcap)

            xt = [[None] * SQT for _ in range(HCHUNKS)]
            for piece in range(SQT):
                for c in range(HCHUNKS):
                    t = xtp.tile([128, QW], bf16, tag=f"xt{c}_{piece}",
                                 name=f"xt{c}_{piece}")
                    xt[c][piece] = t
            wv_sb, wq_sb, wk_sb = [], [], []
            for c in range(HCHUNKS):
                wv_sb.append(const.tile([128, NHL * HS], bf16, tag=f"wv{c}", name=f"wv{c}"))
                wq_sb.append(const.tile([128, NHL * HS], bf16, tag=f"wq{c}", name=f"wq{c}"))
                wk_sb.append(const.tile([128, NHL * HS], bf16, tag=f"wk{c}", name=f"wk{c}"))
            bq_sb = const.tile([128, NHP], fp32, tag="bq")
            bk_sb = const.tile([128, NHP], fp32, tag="bk")
            bv_sb = const.tile([128, NHL * HS], fp32, tag="bv")
            mask_sb = const.tile([128, SKT], fp32, tag="mask")
            wo_sb = [const.tile([64, H], bf16, tag=f"wo{c}", name=f"wo{c}")
                     for c in range(NHL)]

            for c in range(HCHUNKS):
                ld(xt[c][0][:], xt_d[c * 128:(c + 1) * 128, 0:QW])
            ld(bq_sb[:], bq_d[:])
            ld(bk_sb[:], bk_d[:])
            ld(bv_sb[:], bv_d[:])
            ld(mask_sb[:], mask_d[:])
            for c in range(HCHUNKS):
                ld(wv_sb[c][:], wv_d[c * 128:(c + 1) * 128, :])
            for c in range(HCHUNKS):
                ld(xt[c][1][:], xt_d[c * 128:(c + 1) * 128, QW:2 * QW])
            for c in range(HCHUNKS):
                ld(wk_sb[c][:], wk_d[c * 128:(c + 1) * 128, :])
            for c in range(HCHUNKS):
                ld(wq_sb[c][:], wq_d[c * 128:(c + 1) * 128, :])
            for c in range(HCHUNKS):
                ld(xt[c][2][:], xt_d[c * 128:(c + 1) * 128, 2 * QW:3 * QW])
            for c in range(HCHUNKS):
                ld(xt[c][3][:], xt_d[c * 128:(c + 1) * 128, 3 * QW:4 * QW])
            for c in range(NHL):
                ld(wo_sb[c][:], wo_d[c * 64:(c + 1) * 64, :])

            # ---- V projection: V[s, h*65+d], col h*65+64 = 1.0.
            # Emitted as per-kt unit chains so the tail can be injected
            # into the attention loop. ----
            v_sb = [vsb.tile([128, NHL, HS + 1], bf16, tag=f"v{kt}", name=f"v{kt}")
                    for kt in range(SKT)]

            def v_units(kt):
                vt = v_sb[kt]
                st8 = {}

                def unit(c, st8=st8):
                    if c == 0:
                        st8["ps"] = ps_pj.tile([128, 512], fp32, tag="pj",
                                               name="psv")
                    nc.tensor.matmul(
                        st8["ps"][:, :NHL * HS],
                        lhsT=xt[c][kt // 4][:, (kt % 4) * 128:(kt % 4 + 1) * 128],
                        rhs=wv_sb[c][:],
                        start=(c == 0),
                        stop=(c == HCHUNKS - 1),
                    )
                    if c == HCHUNKS - 1:
                        nc.vector.tensor_add(
                            vt[:, :, 0:HS],
                            st8["ps"][:, :NHL * HS].rearrange(
                                "p (h d) -> p h d", h=NHL),
                            bv_sb[:].rearrange("p (h d) -> p h d", h=NHL),
                        )
                        nc.vector.memset(vt[:, :, HS:HS + 1], 1.0)

                return [lambda c=c: unit(c) for c in range(HCHUNKS)]

            for kt in range(8):
                for u in v_units(kt):
                    u()

            combt_a = []
            combt_b = []
            for hp in range(NHP):
                combt_a.append(combtp.tile([64, S], bf16, tag=f"cta{hp}", name=f"cta{hp}"))
                combt_b.append(combtp.tile([64, S], bf16, tag=f"ctb{hp}", name=f"ctb{hp}"))
            # partial output accumulator [128, st, 768]
            out_acc = oaccp.tile([128, S // 128, H], fp32, tag="oacc")

            def emit_qkt(hp):
                """Q^T/K^T projection for head pair hp; returns (qt, kt, units).
                Each unit emits one matmul (plus bias-add drain on the last)."""
                qt_t = qkt.tile([128, S], bf16, tag="qt", name=f"qt{hp}")
                kt_t = qkt.tile([128, S], bf16, tag="kt", name=f"kt{hp}")
                units = []
                chains = {}
                for kind, dst, w_sb, b_sb in (("kt", kt_t, wk_sb, bk_sb),
                                              ("qt", qt_t, wq_sb, bq_sb)):
                    for sq in range(SQT):
                        st8 = {}

                        def unit(c, dst=dst, w_sb=w_sb, b_sb=b_sb, sq=sq, st8=st8):
                            if c == 0:
                                st8["ps"] = ps_pj.tile(
                                    [128, 512], fp32, tag="pj", name="psq")
                            nc.tensor.matmul(
                                st8["ps"][:],
                                lhsT=w_sb[c][:, hp * 128:(hp + 1) * 128],
                                rhs=xt[c][sq][:],
                                start=(c == 0),
                                stop=(c == HCHUNKS - 1),
                            )
                            if c == HCHUNKS - 1:
                                nc.vector.tensor_scalar_add(
                                    dst[:, sq * QW:(sq + 1) * QW], st8["ps"][:],
                                    b_sb[:, hp:hp + 1],
                                )

                        chain = [lambda c=c, u=unit: u(c)
                                 for c in range(HCHUNKS)]
                        chains[(kind, sq)] = chain
                        units.extend(chain)
                return qt_t, kt_t, units, chains

            def emit_outproj_unit(hp, st, half, phase, st8, stream_out=False):
                """One of two matmuls of the out-proj psum chain (st, half)
                for head pair hp; phase 0 = first chunk, 1 = second + drain."""
                if phase == 0:
                    st8["ps"] = ps_pj.tile([128, 512], fp32, tag="pj", name="pso")
                ct = combt_a[hp] if phase == 0 else combt_b[hp]
                nc.tensor.matmul(
                    st8["ps"][:, 0:384],
                    lhsT=ct[:, st * 128:(st + 1) * 128],
                    rhs=wo_sb[2 * hp + phase][:, half * 384:(half + 1) * 384],
                    start=(phase == 0), stop=(phase == 1),
                )
                if phase == 1:
                    dst = out_acc[:, st, half * 384:(half + 1) * 384]
                    if hp == 0:
                        nc.vector.tensor_copy(dst, st8["ps"][:, 0:384])
                    else:
                        nc.vector.tensor_add(dst, dst, st8["ps"][:, 0:384])
                    if stream_out:
                        nc.sync.dma_start(
                            out_d[st * 128:(st + 1) * 128, :], out_acc[:, st, :])

            def outproj_units(hp, sqs, stream_out=False, min_sq=None):
                """(min_sq, unit) out-proj work for the s-tiles inside query
                tiles `sqs` of head pair hp, gated one sq later (or at an
                explicit min_sq when queued into a later head pair)."""
                units = []
                for sq in sqs:
                    gate = sq + 2 if min_sq is None else min_sq
                    for st in range(4 * sq, 4 * (sq + 1)):
                        for half in range(2):
                            st8 = {}
                            for phase in range(2):
                                units.append((gate, lambda hp=hp, st=st,
                                              half=half, phase=phase, st8=st8,
                                              so=stream_out and phase == 1 and half == 1:
                                              emit_outproj_unit(hp, st, half, phase, st8, so)))
                return units

            qkts = [emit_qkt(0)]
            ch0 = qkts[0][3]
            for u in ch0[("kt", 0)] + ch0[("kt", 1)] + ch0[("qt", 0)]:
                u()

            # per-hp injection queues: (min_sq, emit_fn)
            inject_q = {0: [], 1: [], 2: []}
            qkts.append(emit_qkt(1))
            inject_q[0] = (
                [(0, u) for u in ch0[("kt", 2)]]
                + [(0, u) for u in ch0[("kt", 3)]]
                + [(0, u) for u in ch0[("qt", 1)]]
                + [(0, u) for kt in range(8, SKT) for u in v_units(kt)]
                + [(1, u) for u in ch0[("qt", 2)]]
                + [(2, u) for u in ch0[("qt", 3)]]
                + [(0, u) for u in qkts[1][2]]
                + outproj_units(0, range(SQT - 2)))

            slots = [(hp, sq, kt) for hp in range(NHP) for sq in range(SQT)
                     for kt in range(SKT)]

            sc_tiles = {}

            def scores(hp, sq, kt):
                qt_t, kt_t = qkts[hp][0], qkts[hp][1]
                sc = ps_sc.tile([128, 1024], fp32, tag="sc", name="sc")
                nc.tensor.matmul(
                    sc[:, 0:512],
                    lhsT=kt_t[0:64, kt * 128:(kt + 1) * 128],
                    rhs=qt_t[0:64, sq * QW:(sq + 1) * QW],
                    start=True, stop=True,
                )
                nc.tensor.matmul(
                    sc[:, 512:1024],
                    lhsT=kt_t[64:128, kt * 128:(kt + 1) * 128],
                    rhs=qt_t[64:128, sq * QW:(sq + 1) * QW],
                    start=True, stop=True,
                )
                return sc

            cb_cur = None
            sc_cur = scores(*slots[0])
            for i, (hp, sq, kt) in enumerate(slots):
                if kt == 0:
                    if sq == 0 and hp > 0:
                        # drain any leftover injected work of the previous hp
                        for _, u in inject_q[hp - 1]:
                            u()
                        inject_q[hp - 1] = []
                    # build hp-level injection queues lazily at hp start
                    if sq == 0 and hp == 1:
                        qkts.append(emit_qkt(2))
                        inject_q[1] = (outproj_units(0, [SQT - 2, SQT - 1], min_sq=0)
                                       + [(0, u) for u in qkts[2][2]]
                                       + outproj_units(1, range(SQT - 2)))

                    if sq == 0 and hp == 2:
                        inject_q[2] = (outproj_units(1, [SQT - 2, SQT - 1], min_sq=0)
                                       + outproj_units(2, range(SQT - 2),
                                                       stream_out=True)
                                       + outproj_units(2, [SQT - 2],
                                                       stream_out=True,
                                                       min_sq=SQT - 1))
                    cb_a = ps_cb.tile([65, 512], fp32, tag="cb", name="cba")
                    cb_b = ps_cb.tile([65, 512], fp32, tag="cb", name="cbb")
                    cb_cur = (cb_a, cb_b)
                # lookahead scores for the next slot
                sc_nxt = scores(*slots[i + 1]) if i + 1 < len(slots) else None
                at = attnp.tile([128, 1024], bf16, tag="at")
                nc.scalar.activation(
                    at[:], sc_cur[:], AF.Exp,
                    bias=mask_sb[:, kt:kt + 1], scale=0.125,
                )
                # fill the PE exp-wait bubble with independent work
                # (scan past gated units so a blocked head doesn't starve
                # eligible work behind it)
                q = inject_q[hp]
                popped = 0
                max_pop = 5 if (hp == 0 and sq == 0) else 2
                j = 0
                while j < len(q) and popped < max_pop:
                    if q[j][0] <= sq:
                        q.pop(j)[1]()
                        popped += 1
                    else:
                        j += 1
                cb_a, cb_b = cb_cur
                nc.tensor.matmul(
                    cb_a[:],
                    lhsT=v_sb[kt][:, 2 * hp, :],
                    rhs=at[:, 0:512],
                    start=(kt == 0), stop=(kt == SKT - 1),
                )
                nc.tensor.matmul(
                    cb_b[:],
                    lhsT=v_sb[kt][:, 2 * hp + 1, :],
                    rhs=at[:, 512:1024],
                    start=(kt == 0), stop=(kt == SKT - 1),
                )
                sc_cur = sc_nxt
                if kt == SKT - 1:
                    # normalize: comb rows 0..63 / denom(row 64).
                    # Two quick copies free both PSUM banks before the
                    # slow recip/broadcast chains run.
                    cbs_list = []
                    for cb in (cb_a, cb_b):
                        cbs = smallp.tile([65, 512], fp32, tag="cbs", name="cbs")
                        nc.vector.tensor_copy(cbs[:], cb[:])
                        cbs_list.append(cbs)
                    for cbs, ct in ((cbs_list[0], combt_a[hp]),
                                    (cbs_list[1], combt_b[hp])):
                        rc0 = smallp.tile([1, 512], fp32, tag="rc0")
                        nc.sync.dma_start(rc0[:], cbs[64:65, :])
                        rc1 = smallp.tile([1, 512], fp32, tag="rc1")
                        # approx recip is partition-0 only on HW
                        nc.vector.reciprocal_approx_fast(rc1[:], rc0[:])
                        bc = smallp.tile([64, 512], fp32, tag="bc")
                        nc.gpsimd.partition_broadcast(bc[:], rc1[:])
                        nc.vector.tensor_mul(
                            ct[:, sq * QW:(sq + 1) * QW], cbs[0:64, :], bc[:],
                        )

            # ---- tail: leftovers (hp2 out-proj of sq3), streaming out ----
            for hp in range(NHP):
                for _, u in inject_q[hp]:
                    u()
                inject_q[hp] = []
            for st in range(4 * (SQT - 1), 4 * SQT):
                for half in range(2):
                    st8 = {}
                    emit_outproj_unit(2, st, half, 0, st8)
                    emit_outproj_unit(2, st, half, 1, st8,
                                      stream_out=(half == 1))

    nc.compile()
    return nc


def _get_compiled():
    global _COMPILED
    if _COMPILED is None:
        _COMPILED = _build()
    return _COMPILED


def _prep_core_inputs(x, mask, Wq, bq, Wk, bk, Wv, bv, Wo, core):
    b, hg = core // 2, core % 2
    lo, hi = hg * NHL * HS, (hg + 1) * NHL * HS
    bf = ml_dtypes.bfloat16
    return {
        "xt": np.ascontiguousarray(x[b].T).astype(bf),
        "wq": np.ascontiguousarray(Wq[:, lo:hi]).astype(bf),
        "wk": np.ascontiguousarray(Wk[:, lo:hi]).astype(bf),
        "wv": np.ascontiguousarray(Wv[:, lo:hi]).astype(bf),
        "wo": np.ascontiguousarray(Wo[lo:hi, :]).astype(bf),
        "bq": np.ascontiguousarray(bq[lo:hi].reshape(NHP, 128).T).astype(np.float32),
        "bk": np.ascontiguousarray(bk[lo:hi].reshape(NHP, 128).T).astype(np.float32),
        "bv": np.tile(bv[lo:hi][None, :], (128, 1)).astype(np.float32),
        "mask": np.ascontiguousarray(
            mask[b, 0, 0].reshape(SKT, 128).T).astype(np.float32),
    }


def kernel(x, additive_attention_mask, Wq, bq, Wk, bk, Wv, bv, Wo, bo):
    from concourse import bass2jax

    x = np.asarray(x, dtype=np.float32)
    mask = np.asarray(additive_attention_mask, dtype=np.float32)
    args = [np.asarray(a, dtype=np.float32) for a in (Wq, bq, Wk, bk, Wv, bv, Wo)]
    Wq, bq, Wk, bk, Wv, bv, Wo = args
    bo = np.asarray(bo, dtype=np.float32)

    nc = _get_compiled()
    in_maps = [
        _prep_core_inputs(x, mask, Wq, bq, Wk, bk, Wv, bv, Wo, c)
        for c in range(N_CORES)
    ]
    results = bass2jax.run_bass_via_pjrt(nc, in_maps, n_cores=N_CORES)

    out = np.empty((B, S, H), dtype=np.float32)
    for b in range(B):
        out[b] = results[2 * b]["out"] + results[2 * b + 1]["out"] + bo
    return out


# revision 23
# speedup vs baseline: 1.0133x; 1.0133x over previous
"""BERT self-attention (B=4, S=2048, H=768, 12 heads x 64) on 8 trn2 cores.

Sharding: core c = batch (c//2) x head-half (c%2, 6 heads each).
Each core computes Q/K/V projections for its 6 heads, attention, and a
partial output projection (its heads' slice of Wo). Host sums the two
partials per batch and adds bo.

On-device layout (per core):
  xT   [768, 2048]  bf16  (DMA-transposed x)
  QT/KT per head-pair [128=2x64, 2048] bf16   (head-dim on partitions)
  V    16 tiles [128 keys, 6 heads x 65] bf16 (col 64 of each head = 1.0)
  scores^T [128 keys, 2x512 q] fp32 PSUM (two heads packed via row tiling)
  exp on ScalarE (scale=1/8, bias=mask column), out bf16
  attn@V -> comb [65, 512] PSUM; row 64 = softmax denominator
  combT_A/B [64, 2048] bf16, scaled by 1/denom
  out-proj: 6 x K=64 chunks accumulated in SBUF out_acc, fp32

The attention loop is ACT(exp)-bound. The in-order PE is kept busy by
(a) pipelining the score matmuls one iteration ahead (across sq and
head-pair boundaries), and (b) injecting independent projection matmuls
(next pair's Q/K proj, previous tiles' out-proj) into the exp-wait
bubble between scores(i+1) and attn@V(i).
"""

import numpy as np
import ml_dtypes

B, S, H = 4, 2048, 768
NH, HS = 12, 64
NHL = 6              # heads per core
NHP = 3              # head pairs per core
HCHUNKS = 6          # 768 / 128 contraction chunks
SKT = 16             # key tiles of 128
SQT = 4              # query tiles of 512
QW = 512             # query tile width
N_CORES = 8

_COMPILED = None


def _build():
    import concourse.bass as bass
    import concourse.mybir as mybir
    import concourse.tile as tile
    from concourse import bacc

    fp32 = mybir.dt.float32
    bf16 = mybir.dt.bfloat16
    AF = mybir.ActivationFunctionType

    nc = bacc.Bacc("TRN2", target_bir_lowering=False, debug=False)

    xt_d = nc.dram_tensor("xt", [H, S], bf16, kind="ExternalInput").ap()
    wq_d = nc.dram_tensor("wq", [H, NHL * HS], bf16, kind="ExternalInput").ap()
    wk_d = nc.dram_tensor("wk", [H, NHL * HS], bf16, kind="ExternalInput").ap()
    wv_d = nc.dram_tensor("wv", [H, NHL * HS], bf16, kind="ExternalInput").ap()
    wo_d = nc.dram_tensor("wo", [NHL * HS, H], bf16, kind="ExternalInput").ap()
    bq_d = nc.dram_tensor("bq", [128, NHP], fp32, kind="ExternalInput").ap()
    bk_d = nc.dram_tensor("bk", [128, NHP], fp32, kind="ExternalInput").ap()
    bv_d = nc.dram_tensor("bv", [128, NHL * HS], fp32, kind="ExternalInput").ap()
    mask_d = nc.dram_tensor("mask", [128, SKT], fp32, kind="ExternalInput").ap()
    out_d = nc.dram_tensor("out", [S, H], fp32, kind="ExternalOutput").ap()

    with tile.TileContext(nc) as tc:
        with (
            tc.tile_pool(name="const", bufs=1) as const,
            tc.tile_pool(name="xt", bufs=1) as xtp,
            tc.tile_pool(name="vsb", bufs=1) as vsb,
            tc.tile_pool(name="qkt", bufs=2) as qkt,
            tc.tile_pool(name="combt", bufs=1) as combtp,
            tc.tile_pool(name="oacc", bufs=1) as oaccp,
            tc.tile_pool(name="attn", bufs=4) as attnp,
            tc.tile_pool(name="small", bufs=4) as smallp,
            tc.tile_pool(name="ps_sc", bufs=2, space="PSUM") as ps_sc,
            tc.tile_pool(name="ps_cb", bufs=2, space="PSUM") as ps_cb,
            tc.tile_pool(name="ps_pj", bufs=2, space="PSUM") as ps_pj,
        ):
            # ---- x^T (host-transposed) on the sync HWDGE queue, weights
            # in parallel on the scalar HWDGE queue ----
            # x^T as separate [128, 512] piece-tiles so the first V/K
            # chains start as soon as the leading pieces have landed.
            # All startup DMAs are emitted in consumption order, alternating
            # between the two HWDGE queues.
            _dma_i = [0]

            def ld(dst, srcap):
                eng = nc.sync if _dma_i[0] % 2 == 0 else nc.scalar
                _dma_i[0] += 1
                eng.dma_start(dst, srcap)

            xt = [[None] * SQT for _ in range(HCHUNKS)]
            for piece in range(SQT):
                for c in range(HCHUNKS):
                    t = xtp.tile([128, QW], bf16, tag=f"xt{c}_{piece}",
                                 name=f"xt{c}_{piece}")
                    xt[c][piece] = t
            wv_sb, wq_sb, wk_sb = [], [], []
            for c in range(HCHUNKS):
                wv_sb.append(const.tile([128, NHL * HS], bf16, tag=f"wv{c}", name=f"wv{c}"))
                wq_sb.append(const.tile([128, NHL * HS], bf16, tag=f"wq{c}", name=f"wq{c}"))
                wk_sb.append(const.tile([128, NHL * HS], bf16, tag=f"wk{c}", name=f"wk{c}"))
            bq_sb = const.tile([128, NHP], fp32, tag="bq")
            bk_sb = const.tile([128, NHP], fp32, tag="bk")
            bv_sb = const.tile([128, NHL * HS], fp32, tag="bv")
            mask_sb = const.tile([128, SKT], fp32, tag="mask")
            wo_sb = [const.tile([64, H], bf16, tag=f"wo{c}", name=f"wo{c}")
                     for c in range(NHL)]

            for c in range(HCHUNKS):
                ld(xt[c][0][:], xt_d[c * 128:(c + 1) * 128, 0:QW])
            ld(bq_sb[:], bq_d[:])
            ld(bk_sb[:], bk_d[:])
            ld(bv_sb[:], bv_d[:])
            ld(mask_sb[:], mask_d[:])
            for c in range(HCHUNKS):
                ld(wv_sb[c][:], wv_d[c * 128:(c + 1) * 128, :])
            for c in range(HCHUNKS):
                ld(xt[c][1][:], xt_d[c * 128:(c + 1) * 128, QW:2 * QW])
            for c in range(HCHUNKS):
                ld(wk_sb[c][:], wk_d[c * 128:(c + 1) * 128, :])
            for c in range(HCHUNKS):
                ld(wq_sb[c][:], wq_d[c * 128:(c + 1) * 128, :])
            for c in range(HCHUNKS):
                ld(xt[c][2][:], xt_d[c * 128:(c + 1) * 128, 2 * QW:3 * QW])
            for c in range(HCHUNKS):
                ld(xt[c][3][:], xt_d[c * 128:(c + 1) * 128, 3 * QW:4 * QW])
            for c in range(NHL):
                ld(wo_sb[c][:], wo_d[c * 64:(c + 1) * 64, :])

            # ---- V projection: V[s, h*65+d], col h*65+64 = 1.0.
            # Emitted as per-kt unit chains so the tail can be injected
            # into the attention loop. ----
            v_sb = [vsb.tile([128, NHL, HS + 1], bf16, tag=f"v{kt}", name=f"v{kt}")
                    for kt in range(SKT)]

            def v_units(kt):
                vt = v_sb[kt]
                st8 = {}

                def unit(c, st8=st8):
                    if c == 0:
                        st8["ps"] = ps_pj.tile([128, 512], fp32, tag="pj",
                                               name="psv")
                    nc.tensor.matmul(
                        st8["ps"][:, :NHL * HS],
                        lhsT=xt[c][kt // 4][:, (kt % 4) * 128:(kt % 4 + 1) * 128],
                        rhs=wv_sb[c][:],
                        start=(c == 0),
                        stop=(c == HCHUNKS - 1),
                    )
                    if c == HCHUNKS - 1:
                        nc.vector.tensor_add(
                            vt[:, :, 0:HS],
                            st8["ps"][:, :NHL * HS].rearrange(
                                "p (h d) -> p h d", h=NHL),
                            bv_sb[:].rearrange("p (h d) -> p h d", h=NHL),
                        )
                        nc.vector.memset(vt[:, :, HS:HS + 1], 1.0)

                return [lambda c=c: unit(c) for c in range(HCHUNKS)]

            for kt in range(8):
                for u in v_units(kt):
                    u()

            combt_a = []
            combt_b = []
            for hp in range(NHP):
                combt_a.append(combtp.tile([64, S], bf16, tag=f"cta{hp}", name=f"cta{hp}"))
                combt_b.append(combtp.tile([64, S], bf16, tag=f"ctb{hp}", name=f"ctb{hp}"))
            # partial output accumulator [128, st, 768]
            out_acc = oaccp.tile([128, S // 128, H], fp32, tag="oacc")

            def emit_qkt(hp):
                """Q^T/K^T projection for head pair hp; returns (qt, kt, units).
                Each unit emits one matmul (plus bias-add drain on the last)."""
                qt_t = qkt.tile([128, S], bf16, tag="qt", name=f"qt{hp}")
                kt_t = qkt.tile([128, S], bf16, tag="kt", name=f"kt{hp}")
                units = []
                chains = {}
                for kind, dst, w_sb, b_sb in (("kt", kt_t, wk_sb, bk_sb),
                                              ("qt", qt_t, wq_sb, bq_sb)):
                    for sq in range(SQT):
                        st8 = {}

                        def unit(c, dst=dst, w_sb=w_sb, b_sb=b_sb, sq=sq, st8=st8):
                            if c == 0:
                                st8["ps"] = ps_pj.tile(
                                    [128, 512], fp32, tag="pj", name="psq")
                            nc.tensor.matmul(
                                st8["ps"][:],
                                lhsT=w_sb[c][:, hp * 128:(hp + 1) * 128],
                                rhs=xt[c][sq][:],
                                start=(c == 0),
                                stop=(c == HCHUNKS - 1),
                            )
                            if c == HCHUNKS - 1:
                                nc.vector.tensor_scalar_add(
                                    dst[:, sq * QW:(sq + 1) * QW], st8["ps"][:],
                                    b_sb[:, hp:hp + 1],
                                )

                        chain = [lambda c=c, u=unit: u(c)
                                 for c in range(HCHUNKS)]
                        chains[(kind, sq)] = chain
                        units.extend(chain)
                return qt_t, kt_t, units, chains

            def emit_outproj_unit(hp, st, half, phase, st8, stream_out=False):
                """One of two matmuls of the out-proj psum chain (st, half)
                for head pair hp; phase 0 = first chunk, 1 = second + drain."""
                if phase == 0:
                    st8["ps"] = ps_pj.tile([128, 512], fp32, tag="pj", name="pso")
                ct = combt_a[hp] if phase == 0 else combt_b[hp]
                nc.tensor.matmul(
                    st8["ps"][:, 0:384],
                    lhsT=ct[:, st * 128:(st + 1) * 128],
                    rhs=wo_sb[2 * hp + phase][:, half * 384:(half + 1) * 384],
                    start=(phase == 0), stop=(phase == 1),
                )
                if phase == 1:
                    dst = out_acc[:, st, half * 384:(half + 1) * 384]
                    if hp == 0:
                        nc.vector.tensor_copy(dst, st8["ps"][:, 0:384])
                    else:
                        nc.vector.tensor_add(dst, dst, st8["ps"][:, 0:384])
                    if stream_out:
                        nc.sync.dma_start(
                            out_d[st * 128:(st + 1) * 128, :], out_acc[:, st, :])

            def outproj_units(hp, sqs, stream_out=False, min_sq=None):
                """(min_sq, unit) out-proj work for the s-tiles inside query
                tiles `sqs` of head pair hp, gated one sq later (or at an
                explicit min_sq when queued into a later head pair)."""
                units = []
                for sq in sqs:
                    gate = sq + 2 if min_sq is None else min_sq
                    for st in range(4 * sq, 4 * (sq + 1)):
                        for half in range(2):
                            st8 = {}
                            for phase in range(2):
                                units.append((gate, lambda hp=hp, st=st,
                                              half=half, phase=phase, st8=st8,
                                              so=stream_out and phase == 1 and half == 1:
                                              emit_outproj_unit(hp, st, half, phase, st8, so)))
                return units

            qkts = [emit_qkt(0)]
            ch0 = qkts[0][3]
            for u in ch0[("kt", 0)] + ch0[("kt", 1)] + ch0[("qt", 0)]:
                u()

            # per-hp injection queues: (min_sq, emit_fn)
            inject_q = {0: [], 1: [], 2: []}
            qkts.append(emit_qkt(1))
            inject_q[0] = (
                [(0, u) for u in ch0[("kt", 2)]]
                + [(0, u) for u in ch0[("kt", 3)]]
                + [(0, u) for u in ch0[("qt", 1)]]
                + [(0, u) for kt in range(8, SKT) for u in v_units(kt)]
                + [(1, u) for u in ch0[("qt", 2)]]
                + [(2, u) for u in ch0[("qt", 3)]]
                + [(0, u) for u in qkts[1][2]]
                + outproj_units(0, range(SQT - 2)))

            slots = [(hp, sq, kt) for hp in range(NHP) for sq in range(SQT)
                     for kt in range(SKT)]

            sc_tiles = {}

            def scores(hp, sq, kt):
                qt_t, kt_t = qkts[hp][0], qkts[hp][1]
                sc = ps_sc.tile([128, 1024], fp32, tag="sc", name="sc")
                nc.tensor.matmul(
                    sc[:, 0:512],
                    lhsT=kt_t[0:64, kt * 128:(kt + 1) * 128],
                    rhs=qt_t[0:64, sq * QW:(sq + 1) * QW],
                    start=True, stop=True,
                )
                nc.tensor.matmul(
                    sc[:, 512:1024],
                    lhsT=kt_t[64:128, kt * 128:(kt + 1) * 128],
                    rhs=qt_t[64:128, sq * QW:(sq + 1) * QW],
                    start=True, stop=True,
                )
                return sc

            cb_cur = None
            sc_cur = scores(*slots[0])
            for i, (hp, sq, kt) in enumerate(slots):
                if kt == 0:
                    if sq == 0 and hp > 0:
                        # drain any leftover injected work of the previous hp
                        for _, u in inject_q[hp - 1]:
                            u()
                        inject_q[hp - 1] = []
                    # build hp-level injection queues lazily at hp start
                    if sq == 0 and hp == 1:
                        qkts.append(emit_qkt(2))
                        inject_q[1] = (outproj_units(0, [SQT - 2, SQT - 1], min_sq=0)
                                       + [(0, u) for u in qkts[2][2]]
                                       + outproj_units(1, range(SQT - 2)))

                    if sq == 0 and hp == 2:
                        inject_q[2] = (outproj_units(1, [SQT - 2, SQT - 1], min_sq=0)
                                       + outproj_units(2, range(SQT - 2),
                                                       stream_out=True)
                                       + outproj_units(2, [SQT - 2],
                                                       stream_out=True,
                                                       min_sq=SQT - 1))
                    cb_a = ps_cb.tile([65, 512], fp32, tag="cb", name="cba")
                    cb_b = ps_cb.tile([65, 512], fp32, tag="cb", name="cbb")
                    cb_cur = (cb_a, cb_b)
                # lookahead scores for the next slot
                sc_nxt = scores(*slots[i + 1]) if i + 1 < len(slots) else None
                at = attnp.tile([128, 1024], bf16, tag="at")
                nc.scalar.activation(
                    at[:], sc_cur[:], AF.Exp,
                    bias=mask_sb[:, kt:kt + 1], scale=0.125,
                )
                # fill the PE exp-wait bubble with independent work
                # (scan past gated units so a blocked head doesn't starve
                # eligible work behind it)
                q = inject_q[hp]
                popped = 0
                max_pop = 5 if (hp == 0 and sq == 0) else 2
                j = 0
                while j < len(q) and popped < max_pop:
                    if q[j][0] <= sq:
                        q.pop(j)[1]()
                        popped += 1
                    else:
                        j += 1
                cb_a, cb_b = cb_cur
                nc.tensor.matmul(
                    cb_a[:],
                    lhsT=v_sb[kt][:, 2 * hp, :],
                    rhs=at[:, 0:512],
                    start=(kt == 0), stop=(kt == SKT - 1),
                )
                nc.tensor.matmul(
                    cb_b[:],
                    lhsT=v_sb[kt][:, 2 * hp + 1, :],
                    rhs=at[:, 512:1024],
                    start=(kt == 0), stop=(kt == SKT - 1),
                )
                sc_cur = sc_nxt
                if kt == SKT - 1:
                    # normalize: comb rows 0..63 / denom(row 64).
                    # Two quick copies free both PSUM banks before the
                    # slow recip/broadcast chains run.
                    cbs_list = []
                    for cb in (cb_a, cb_b):
                        cbs = smallp.tile([65, 512], fp32, tag="cbs", name="cbs")
                        nc.vector.tensor_copy(cbs[:], cb[:])
                        cbs_list.append(cbs)
                    for cbs, ct in ((cbs_list[0], combt_a[hp]),
                                    (cbs_list[1], combt_b[hp])):
                        rc0 = smallp.tile([1, 512], fp32, tag="rc0")
                        nc.sync.dma_start(rc0[:], cbs[64:65, :])
                        rc1 = smallp.tile([1, 512], fp32, tag="rc1")
                        # approx recip is partition-0 only on HW
                        nc.vector.reciprocal_approx_fast(rc1[:], rc0[:])
                        bc = smallp.tile([64, 512], fp32, tag="bc")
                        nc.gpsimd.partition_broadcast(bc[:], rc1[:])
                        nc.vector.tensor_mul(
                            ct[:, sq * QW:(sq + 1) * QW], cbs[0:64, :], bc[:],
                        )

            # ---- tail: leftovers (hp2 out-proj of sq3), streaming out ----
            for hp in range(NHP):
                for _, u in inject_q[hp]:
                    u()
                inject_q[hp] = []
            for st in range(4 * (SQT - 1), 4 * SQT):
                for half in range(2):
                    st8 = {}
                    emit_outproj_unit(2, st, half, 0, st8)
                    emit_outproj_unit(2, st, half, 1, st8,
                                      stream_out=(half == 1))

    nc.compile()
    return nc


def _get_compiled():
    global _COMPILED
    if _COMPILED is None:
        _COMPILED = _build()
    return _COMPILED


def _prep_core_inputs(x, mask, Wq, bq, Wk, bk, Wv, bv, Wo, core):
    b, hg = core // 2, core % 2
    lo, hi = hg * NHL * HS, (hg + 1) * NHL * HS
    bf = ml_dtypes.bfloat16
    return {
        "xt": np.ascontiguousarray(x[b].T).astype(bf),
        "wq": np.ascontiguousarray(Wq[:, lo:hi]).astype(bf),
        "wk": np.ascontiguousarray(Wk[:, lo:hi]).astype(bf),
        "wv": np.ascontiguousarray(Wv[:, lo:hi]).astype(bf),
        "wo": np.ascontiguousarray(Wo[lo:hi, :]).astype(bf),
        "bq": np.ascontiguousarray(bq[lo:hi].reshape(NHP, 128).T).astype(np.float32),
        "bk": np.ascontiguousarray(bk[lo:hi].reshape(NHP, 128).T).astype(np.float32),
        "bv": np.tile(bv[lo:hi][None, :], (128, 1)).astype(np.float32),
        "mask": np.ascontiguousarray(
            mask[b, 0, 0].reshape(SKT, 128).T).astype(np.float32),
    }


def kernel(x, additive_attention_mask, Wq, bq, Wk, bk, Wv, bv, Wo, bo):
    from concourse import bass2jax

    x = np.asarray(x, dtype=np.float32)
    mask = np.asarray(additive_attention_mask, dtype=np.float32)
    args = [np.asarray(a, dtype=np.float32) for a in (Wq, bq, Wk, bk, Wv, bv, Wo)]
    Wq, bq, Wk, bk, Wv, bv, Wo = args
    bo = np.asarray(bo, dtype=np.float32)

    nc = _get_compiled()
    in_maps = [
        _prep_core_inputs(x, mask, Wq, bq, Wk, bk, Wv, bv, Wo, c)
        for c in range(N_CORES)
    ]
    results = bass2jax.run_bass_via_pjrt(nc, in_maps, n_cores=N_CORES)

    out = np.empty((B, S, H), dtype=np.float32)
    for b in range(B):
        out[b] = results[2 * b]["out"] + results[2 * b + 1]["out"] + bo
    return out
